# revision 29
# baseline (speedup 1.0000x reference)
"""Trainium2 Bass kernel for batched 8x8-block 2D DCT.

Input  x: (32, 3, 512, 512) f32, dct_basis: (8, 8) f32.
Output y: (32, 3, 512, 512) f32 with each 8x8 block B replaced by D @ B @ D^T.

Sharding: data-parallel over the batch dim - 32 batches -> 8 NeuronCores x 4.
Each core runs an identical (SPMD) Bass program over its (4,3,512,512) slice,
viewed as a [6144, 512] row-major matrix = 24 supertiles of [128, 1024]
(256 image rows x 512 cols; partition p = row within a 128-row band, free
dim = (band t in {0,1}, col w)).

Shipped dataflow (mode "ff2r" - fused-transpose, all fp32):
  per [128,128] chunk X_c, both DCT passes run on the Tensor engine with the
  DATA as the stationary operand and Bblk^T = kron(I_16, D)^T as the moving
  operand:  PE(out, lhsT=X_c, rhs=Bblk^T) = X_c^T @ Bblk^T = (Bblk X_c)^T.
  Each pass applies the 8-point DCT along the contracted (partition) axis
  AND transposes the chunk, so two passes return to the original
  orientation - no DVE stream transposes and no transpose DMAs at all.
  PSUM->SBUF copies are balanced across the Scalar and Vector engines.
  Input DMAs ride the SP HWDGE ring, output DMAs the ACT HWDGE ring
  (decoupled FIFOs), constants the idle SWDGE ring. Data DMAs are 1 MiB
  ([128, 2048] supertiles, 4 image-row bands each); a mini-tile ladder at
  the kernel ends shortens pipeline fill/drain. The problem is HBM-bound:
  per core 12 MiB in + 12 MiB out; measured pure-DMA floor ~75-82 us/rep,
  this kernel ~88-91 us steady-state (same-round floor +7-10 us).

Other modes (v2p*, rampopt, dma*, ...) are kept as probe/baseline variants;
see _build_nc.
"""

import sys

for _p in ("/opt/trn_rl_repo",):
    if _p not in sys.path:
        sys.path.insert(0, _p)

from contextlib import ExitStack

import numpy as np

N_CORES = 8
B, C, H, W = 32, 3, 512, 512
ROWS_PER_CORE = (B // N_CORES) * C * H  # 6144
N_SUPER = ROWS_PER_CORE // 256  # 24

_NC_CACHE = {}


def _build_nc(rep=1, use_f32r=False, psum_transpose=False, mode="full"):
    import concourse.bacc as bacc
    import concourse.tile as tile
    import concourse.mybir as mybir

    F32 = mybir.dt.float32
    F32R = mybir.dt.float32r

    FIN = F32R if use_f32r else F32

    nc = bacc.Bacc(
        "TRN2",
        target_bir_lowering=False,
        debug=False,
        enable_asserts=False,
    )
    x_ap = nc.dram_tensor("x", [ROWS_PER_CORE, 512], FIN, kind="ExternalInput").ap()
    bt_ap = nc.dram_tensor("bt", [128, 128], F32, kind="ExternalInput").ap()
    btr_ap = (nc.dram_tensor("btr", [128, 128], F32R, kind="ExternalInput").ap()
              if use_f32r else None)
    BF16 = mybir.dt.bfloat16
    if mode in ("fused", "fd"):
        bth_ap = nc.dram_tensor("bth", [128, 128], BF16, kind="ExternalInput").ap()
        btl_ap = nc.dram_tensor("btl", [128, 128], BF16, kind="ExternalInput").ap()
    y_ap = nc.dram_tensor("y", [ROWS_PER_CORE, 512], F32, kind="ExternalOutput").ap()

    with tile.TileContext(nc) as tc, ExitStack() as ctx:
        xv = x_ap.rearrange("(n t p) w -> n p t w", t=2, p=128)
        yv = y_ap.rearrange("(n t p) w -> n p t w", t=2, p=128)

        def as3d(sb_ap):
            return sb_ap.rearrange("p (t w) -> p t w", t=2)

        const = ctx.enter_context(tc.tile_pool(name="const", bufs=1))
        bt = const.tile([128, 128], F32)
        # constants ride the idle SWDGE ring so the SP HWDGE ring starts on
        # the first data tile immediately
        nc.gpsimd.dma_start(bt[:], bt_ap)
        if use_f32r:
            btr = const.tile([128, 128], F32R)
            nc.gpsimd.dma_start(btr[:], btr_ap)

        nb = 4 if mode in ("tuned", "rampopt", "swin",
                           "v2", "v2p", "v2g", "v3", "v3g", "fd") else 3
        if mode in ("ff", "ffr", "ffx", "ffa"):
            nb = 6
        if mode == "ffb":
            nb = 10
        if mode in ("ff2",):
            nb = 4
        if mode in ("ff2b", "ff2r"):
            nb = 6
        if mode in ("dma", "dma2", "dma4", "dmadual", "dmadual0"):
            nb = 6
        if mode == "dmaph":
            nb = N_SUPER
        if mode in ("v2b", "v3b", "v2pb", "v2pg", "v2ps"):
            nb = 6
        if mode == "v2pc":
            nb = 8
        in_dma = (nc.gpsimd.dma_start if mode in ("swin", "v2g", "v3g", "v2pg")
                  else nc.scalar.dma_start if mode in ("v3", "v3b")
                  else nc.sync.dma_start)
        out_dma = (nc.scalar.dma_start
                   if mode in ("tuned", "v2", "v2b", "v2p", "v2pb", "v2pc",
                               "v2pg", "v2ps", "fd", "ff", "ffr", "ffx",
                               "ffb", "ff2", "ff2b", "ff2r", "ffa")
                   else nc.sync.dma_start)
        if mode in ("v2p", "v2pb", "v2pc", "v2pg", "v2ps"):
            psum_transpose = True
        psum_split = mode == "v2ps"
        xp = ctx.enter_context(tc.tile_pool(name="xp", bufs=nb))
        tp = ctx.enter_context(tc.tile_pool(name="tp", bufs=nb))
        yp = ctx.enter_context(tc.tile_pool(name="yp", bufs=nb))
        psb = 4 if mode in ("fused", "fd") else 2
        pst = ctx.enter_context(tc.tile_pool(name="pst", bufs=psb, space="PSUM"))
        psy = ctx.enter_context(tc.tile_pool(name="psy", bufs=psb, space="PSUM"))
        cpp = ctx.enter_context(tc.tile_pool(name="cpp", bufs=nb))

        lhsT1 = btr[:] if use_f32r else bt[:]
        lhsT2 = bt[:]

        if mode in ("fused", "fd"):
            # Fused-transpose dataflow: data chunks are the STATIONARY
            # operand (fp32, full precision); the moving operand is the
            # basis split hi/lo into bf16 (1 cyc/row) and accumulated in
            # PSUM: out = X_c^T @ (Bth + Btl). Two such matmul pairs per
            # chunk implement both DCT passes with the transposes absorbed
            # by lhsT.T semantics. No DVE stream transposes needed.
            bth = const.tile([128, 128], BF16)
            nc.gpsimd.dma_start(bth[:], bth_ap)
            btl = const.tile([128, 128], BF16)
            nc.gpsimd.dma_start(btl[:], btl_ap)
            for _ in range(rep):
                for s in range(N_SUPER):
                    xs = xp.tile([128, 1024], F32)
                    in_dma(as3d(xs[:]), xv[s])
                    t1 = tp.tile([128, 1024], F32)
                    for b in range(2):
                        pt = pst.tile([128, 512], F32)
                        for q in range(4):
                            c = b * 4 + q
                            for rhs_t, st in ((bth, True), (btl, False)):
                                nc.tensor.matmul(
                                    pt[:, q * 128:(q + 1) * 128],
                                    xs[:, c * 128:(c + 1) * 128],
                                    rhs_t[:],
                                    start=st, stop=not st,
                                    skip_group_check=True,
                                )
                        nc.scalar.copy(t1[:, b * 512:(b + 1) * 512], pt[:])
                    ys = yp.tile([128, 1024], F32)
                    for b in range(2):
                        py = psy.tile([128, 512], F32)
                        for q in range(4):
                            c = b * 4 + q
                            for rhs_t, st in ((bth, True), (btl, False)):
                                nc.tensor.matmul(
                                    py[:, q * 128:(q + 1) * 128],
                                    t1[:, c * 128:(c + 1) * 128],
                                    rhs_t[:],
                                    start=st, stop=not st,
                                    skip_group_check=True,
                                )
                        if b == 0:
                            nc.scalar.copy(ys[:, :512], py[:])
                        else:
                            nc.vector.tensor_copy(ys[:, 512:], py[:])
                    out_dma(yv[s], as3d(ys[:]))
            rep = 0  # skip main loop below

        if mode in ("ff", "ffr", "ffx", "ffb", "ffa"):
            # Fused-transpose dataflow, all-fp32: data chunks [128,128] are
            # the STATIONARY operand, the basis bt (kron(I,D)^T) the MOVING
            # operand for BOTH passes. out = X_c^T @ bt transposes each
            # chunk while applying the DCT along the contracted axis, so the
            # two passes come back to the original orientation with no DVE
            # stream transposes. PSUM->SBUF copies balanced ACT/DVE.
            # ffr: + mini-tile ladder at the kernel ends (shorter fill/drain)
            # ffx: ffr + in/out rings alternate per supertile
            def ff_full(s, ind, outd):
                xs = xp.tile([128, 1024], F32)
                ind(as3d(xs[:]), xv[s])
                t1 = tp.tile([128, 1024], F32)
                for b in range(2):
                    pt = pst.tile([128, 512], F32)
                    for q in range(4):
                        c = b * 4 + q
                        nc.tensor.matmul(
                            pt[:, q * 128:(q + 1) * 128],
                            xs[:, c * 128:(c + 1) * 128],
                            bt[:], start=True, stop=True,
                        )
                    sl = t1[:, b * 512:(b + 1) * 512]
                    if mode == "ffa":
                        # all t1 copies on DVE; ACT reserved for ys + out-DMA
                        nc.vector.tensor_copy(sl, pt[:])
                    elif b == 0:
                        nc.scalar.copy(sl, pt[:])
                    else:
                        nc.vector.tensor_copy(sl, pt[:])
                ys = yp.tile([128, 1024], F32)
                for b in range(2):
                    py = psy.tile([128, 512], F32)
                    for q in range(4):
                        c = b * 4 + q
                        nc.tensor.matmul(
                            py[:, q * 128:(q + 1) * 128],
                            t1[:, c * 128:(c + 1) * 128],
                            bt[:], start=True, stop=True,
                        )
                    sl = ys[:, b * 512:(b + 1) * 512]
                    if mode == "ffa":
                        # ys copies on ACT so out-DMA's wait is queue-local
                        nc.scalar.copy(sl, py[:])
                    elif b == 0:
                        nc.scalar.copy(sl, py[:])
                    else:
                        nc.vector.tensor_copy(sl, py[:])
                outd(yv[s], as3d(ys[:]))

            def ff_mini(s, t, w0, w1, ind, outd):
                wd = w1 - w0
                xs = xp.tile([128, wd], F32)
                ind(xs[:], xv[s][:, t, w0:w1])
                t1 = tp.tile([128, wd], F32)
                pt = pst.tile([128, wd], F32)
                for q in range(wd // 128):
                    nc.tensor.matmul(
                        pt[:, q * 128:(q + 1) * 128],
                        xs[:, q * 128:(q + 1) * 128],
                        bt[:], start=True, stop=True,
                    )
                nc.scalar.copy(t1[:], pt[:])
                ys = yp.tile([128, wd], F32)
                py = psy.tile([128, wd], F32)
                for q in range(wd // 128):
                    nc.tensor.matmul(
                        py[:, q * 128:(q + 1) * 128],
                        t1[:, q * 128:(q + 1) * 128],
                        bt[:], start=True, stop=True,
                    )
                nc.vector.tensor_copy(ys[:], py[:])
                outd(yv[s][:, t, w0:w1], ys[:])

            ff_ladder = mode in ("ffr", "ffx")
            for r in range(rep):
                for s in range(N_SUPER):
                    if mode == "ffx" and s % 2:
                        ind, outd = nc.scalar.dma_start, nc.sync.dma_start
                    else:
                        ind, outd = in_dma, out_dma
                    at_end = ((r == 0 and s <= 1)
                              or (r == rep - 1 and s >= N_SUPER - 2))
                    if ff_ladder and at_end:
                        if s in (0, N_SUPER - 1):
                            for t in range(2):
                                for qq in range(2):
                                    ff_mini(s, t, qq * 256, (qq + 1) * 256,
                                            ind, outd)
                        else:
                            for t in range(2):
                                ff_mini(s, t, 0, 512, ind, outd)
                    else:
                        ff_full(s, ind, outd)
            rep = 0  # skip main loop below

        if mode in ("ff2", "ff2b", "ff2r"):
            # ff dataflow with 1 MiB transfers: [128, 2048] supertiles
            # (4 bands), per-band fused two-pass chunks, copies ACT/DVE
            # alternating per band. ff2r adds the end ladder.
            xv4 = x_ap.rearrange("(n t p) w -> n p t w", t=4, p=128)
            yv4 = y_ap.rearrange("(n t p) w -> n p t w", t=4, p=128)

            def ff2_full(sp):
                xs = xp.tile([128, 2048], F32)
                in_dma(xs[:].rearrange("p (t w) -> p t w", t=4), xv4[sp])
                t1 = tp.tile([128, 2048], F32)
                for g in range(4):
                    pt = pst.tile([128, 512], F32)
                    for q in range(4):
                        c = g * 4 + q
                        nc.tensor.matmul(
                            pt[:, q * 128:(q + 1) * 128],
                            xs[:, c * 128:(c + 1) * 128],
                            bt[:], start=True, stop=True,
                        )
                    if g % 2 == 0:
                        nc.scalar.copy(t1[:, g * 512:(g + 1) * 512], pt[:])
                    else:
                        nc.vector.tensor_copy(
                            t1[:, g * 512:(g + 1) * 512], pt[:])
                ys = yp.tile([128, 2048], F32)
                for g in range(4):
                    py = psy.tile([128, 512], F32)
                    for q in range(4):
                        c = g * 4 + q
                        nc.tensor.matmul(
                            py[:, q * 128:(q + 1) * 128],
                            t1[:, c * 128:(c + 1) * 128],
                            bt[:], start=True, stop=True,
                        )
                    if g % 2 == 0:
                        nc.scalar.copy(ys[:, g * 512:(g + 1) * 512], py[:])
                    else:
                        nc.vector.tensor_copy(
                            ys[:, g * 512:(g + 1) * 512], py[:])
                out_dma(yv4[sp], ys[:].rearrange("p (t w) -> p t w", t=4))

            def ff2_mini(sp, t, half):
                # one [128, 512] band-half of 2-band supertile sp
                s2, tt = divmod(sp * 4 + t, 2)
                xs = xp.tile([128, 512], F32)
                in_dma(xs[:], xv[s2][:, tt, :])
                t1 = tp.tile([128, 512], F32)
                pt = pst.tile([128, 512], F32)
                for q in range(4):
                    nc.tensor.matmul(
                        pt[:, q * 128:(q + 1) * 128],
                        xs[:, q * 128:(q + 1) * 128],
                        bt[:], start=True, stop=True,
                    )
                nc.scalar.copy(t1[:], pt[:])
                ys = yp.tile([128, 512], F32)
                py = psy.tile([128, 512], F32)
                for q in range(4):
                    nc.tensor.matmul(
                        py[:, q * 128:(q + 1) * 128],
                        t1[:, q * 128:(q + 1) * 128],
                        bt[:], start=True, stop=True,
                    )
                nc.vector.tensor_copy(ys[:], py[:])
                out_dma(yv[s2][:, tt, :], ys[:])

            NSP = N_SUPER // 2  # 12 supertiles of [128, 2048]
            for r in range(rep):
                for sp in range(NSP):
                    at_end = ((r == 0 and sp == 0)
                              or (r == rep - 1 and sp == NSP - 1))
                    if mode == "ff2r" and at_end:
                        for t in range(4):
                            ff2_mini(sp, t, 0)
                    else:
                        ff2_full(sp)
            rep = 0  # skip main loop below

        if mode == "bigload":
            # steady-state probe: 1 MiB input DMAs (two supertiles per load),
            # compute pipeline and 512 KiB output DMAs unchanged
            xv4 = x_ap.rearrange("(n t p) w -> n p t w", t=4, p=128)
            for _ in range(rep):
                for sp in range(N_SUPER // 2):
                    xs2 = xp.tile([128, 2048], F32)
                    nc.sync.dma_start(
                        xs2[:].rearrange("p (t w) -> p t w", t=4), xv4[sp])
                    for g in range(2):
                        s = sp * 2 + g
                        xsv = xs2[:, g * 1024:(g + 1) * 1024]
                        pt = pst.tile([128, 1024], F32)
                        for h in range(2):
                            nc.tensor.matmul(
                                pt[:, h * 512:(h + 1) * 512], lhsT2,
                                xsv[:, h * 512:(h + 1) * 512],
                                start=True, stop=True)
                        tc_ = cpp.tile([128, 1024], F32)
                        nc.scalar.copy(tc_[:], pt[:])
                        t1 = tp.tile([128, 1024], F32)
                        nc.vector.transpose(t1[:], tc_[:])
                        py = psy.tile([128, 1024], F32)
                        for h in range(2):
                            nc.tensor.matmul(
                                py[:, h * 512:(h + 1) * 512], lhsT2,
                                t1[:, h * 512:(h + 1) * 512],
                                start=True, stop=True)
                        yc = cpp.tile([128, 1024], F32)
                        nc.scalar.copy(yc[:], py[:])
                        ys = yp.tile([128, 1024], F32)
                        nc.vector.transpose(ys[:], yc[:])
                        nc.sync.dma_start(yv[s], as3d(ys[:]))
            rep = 0  # skip main loop below

        if mode == "full2":
            # [128, 2048] supertiles: 1 MiB DMA transfers, compute in
            # [128, 1024] halves (PSUM: 2+2 banks x2 pools = 8 banks).
            xv4 = x_ap.rearrange("(n t p) w -> n p t w", t=4, p=128)
            yv4 = y_ap.rearrange("(n t p) w -> n p t w", t=4, p=128)
            for _ in range(rep):
                for s in range(N_SUPER // 2):
                    xs = xp.tile([128, 2048], FIN)
                    nc.sync.dma_start(
                        xs[:].rearrange("p (t w) -> p t w", t=4), xv4[s])
                    ys = yp.tile([128, 2048], F32)
                    for g in range(2):
                        pt = pst.tile([128, 1024], F32)
                        for h in range(2):
                            nc.tensor.matmul(
                                pt[:, h * 512:(h + 1) * 512],
                                lhsT1,
                                xs[:, g * 1024 + h * 512:
                                   g * 1024 + (h + 1) * 512],
                                start=True, stop=True,
                            )
                        tc_ = cpp.tile([128, 1024], F32)
                        nc.scalar.copy(tc_[:], pt[:])
                        t1 = tp.tile([128, 1024], F32)
                        nc.vector.transpose(t1[:], tc_[:])
                        py = psy.tile([128, 1024], F32)
                        for h in range(2):
                            nc.tensor.matmul(
                                py[:, h * 512:(h + 1) * 512],
                                lhsT2,
                                t1[:, h * 512:(h + 1) * 512],
                                start=True, stop=True,
                            )
                        yc = cpp.tile([128, 1024], F32)
                        nc.scalar.copy(yc[:], py[:])
                        nc.vector.transpose(
                            ys[:, g * 1024:(g + 1) * 1024], yc[:])
                    nc.sync.dma_start(
                        yv4[s], ys[:].rearrange("p (t w) -> p t w", t=4))
            rep = 0  # skip main loop below

        if mode == "dma2":
            # 1 MiB transfers: [128, 2048] supertiles (4 bands each)
            xv4 = x_ap.rearrange("(n t p) w -> n p t w", t=4, p=128)
            yv4 = y_ap.rearrange("(n t p) w -> n p t w", t=4, p=128)
            for _ in range(rep):
                for s in range(N_SUPER // 2):
                    xs = xp.tile([128, 2048], FIN)
                    nc.sync.dma_start(
                        xs[:].rearrange("p (t w) -> p t w", t=4), xv4[s])
                    nc.sync.dma_start(
                        yv4[s], xs[:].rearrange("p (t w) -> p t w", t=4))
            rep = 0  # skip main loop below

        if mode == "dma4":
            # 2 MiB transfers: [128, 4096] supertiles (8 bands each)
            xv8 = x_ap.rearrange("(n t p) w -> n p t w", t=8, p=128)
            yv8 = y_ap.rearrange("(n t p) w -> n p t w", t=8, p=128)
            for _ in range(rep):
                for s in range(N_SUPER // 4):
                    xs = xp.tile([128, 4096], FIN)
                    nc.sync.dma_start(
                        xs[:].rearrange("p (t w) -> p t w", t=8), xv8[s])
                    nc.sync.dma_start(
                        yv8[s], xs[:].rearrange("p (t w) -> p t w", t=8))
            rep = 0  # skip main loop below

        if mode == "dmadual":
            # 1 MiB transfers, in on SP HWDGE ring, out on ACT HWDGE ring
            xv4 = x_ap.rearrange("(n t p) w -> n p t w", t=4, p=128)
            yv4 = y_ap.rearrange("(n t p) w -> n p t w", t=4, p=128)
            for _ in range(rep):
                for s in range(N_SUPER // 2):
                    xs = xp.tile([128, 2048], FIN)
                    nc.sync.dma_start(
                        xs[:].rearrange("p (t w) -> p t w", t=4), xv4[s])
                    nc.scalar.dma_start(
                        yv4[s], xs[:].rearrange("p (t w) -> p t w", t=4))
            rep = 0  # skip main loop below

        if mode == "dmadual0":
            # 512 KiB transfers, in on SP ring, out on ACT ring
            for _ in range(rep):
                for s in range(N_SUPER):
                    xs = xp.tile([128, 1024], FIN)
                    nc.sync.dma_start(as3d(xs[:]), xv[s])
                    nc.scalar.dma_start(yv[s], as3d(xs[:]))
            rep = 0  # skip main loop below

        if mode == "dmaph":
            # phase-separated: read all 24 supertiles into SBUF, then write
            # all 24 back — measures HBM R/W turnaround cost vs interleaved
            for _ in range(rep):
                tiles = []
                for s in range(N_SUPER):
                    xs = xp.tile([128, 1024], FIN)
                    nc.sync.dma_start(as3d(xs[:]), xv[s])
                    tiles.append(xs)
                for s in range(N_SUPER):
                    nc.scalar.dma_start(yv[s], as3d(tiles[s][:]))
            rep = 0  # skip main loop below

        def mini_super(s, t, w0, w1):
            # [128, w1-w0] slice of band t as its own mini-pipeline; used at
            # the kernel ends to shorten pipeline fill and drain
            wd = w1 - w0
            xs = xp.tile([128, wd], FIN)
            in_dma(xs[:], xv[s][:, t, w0:w1])
            pt = pst.tile([128, wd], F32)
            nc.tensor.matmul(pt[:], lhsT1, xs[:], start=True, stop=True)
            tc_ = cpp.tile([128, wd], F32)
            nc.scalar.copy(tc_[:], pt[:])
            t1 = tp.tile([128, wd], F32)
            nc.vector.transpose(t1[:], tc_[:])
            py = psy.tile([128, wd], F32)
            nc.tensor.matmul(py[:], lhsT2, t1[:], start=True, stop=True)
            yc = cpp.tile([128, wd], F32)
            nc.scalar.copy(yc[:], py[:])
            ys = yp.tile([128, wd], F32)
            nc.vector.transpose(ys[:], yc[:])
            out_dma(yv[s][:, t, w0:w1], ys[:])

        # granularity ladder per supertile index: list of (t, w0, w1) items,
        # or None for the standard full-width path
        def ladder(s):
            if s in (0, N_SUPER - 1):
                items = [(t, q * 256, (q + 1) * 256)
                         for t in range(2) for q in range(2)]
                return items
            if s in (1, N_SUPER - 2):
                return [(0, 0, 512), (1, 0, 512)]
            return None

        split_ends = mode in ("rampopt", "swin", "v2", "v2b", "v2p", "v2g",
                              "v3", "v3b", "v3g", "v2pb", "v2pc", "v2pg",
                              "v2ps")
        for r in range(rep):
            for s in range(N_SUPER):
                # ladder only at the true kernel ends (first/last rep), so
                # rep>1 timing builds measure pure steady-state in between;
                # for rep=1 this is the same program as before
                at_end = (r == 0 and s <= 1) or (r == rep - 1 and s >= N_SUPER - 2)
                items = ladder(s) if (split_ends and at_end) else None
                if items is not None:
                    for (t, w0, w1) in items:
                        mini_super(s, t, w0, w1)
                    continue
                xs = xp.tile([128, 1024], FIN)
                in_dma(as3d(xs[:]), xv[s])

                if mode == "dma":
                    nc.sync.dma_start(yv[s], as3d(xs[:]))
                    continue

                t1 = tp.tile([128, 1024], F32)
                if psum_split:
                    for h in range(2):
                        ph = pst.tile([128, 512], F32)
                        nc.tensor.matmul(ph[:], lhsT1,
                                         xs[:, h * 512:(h + 1) * 512],
                                         start=True, stop=True)
                        nc.vector.transpose(t1[:, h * 512:(h + 1) * 512], ph[:])
                else:
                    pt = pst.tile([128, 1024], F32)
                    for h in range(2):
                        nc.tensor.matmul(
                            pt[:, h * 512:(h + 1) * 512],
                            lhsT1,
                            xs[:, h * 512:(h + 1) * 512],
                            start=True, stop=True,
                        )
                    if psum_transpose:
                        nc.vector.transpose(t1[:], pt[:])
                    else:
                        tc_ = cpp.tile([128, 1024], F32)
                        nc.scalar.copy(tc_[:], pt[:])
                        nc.vector.transpose(t1[:], tc_[:])

                ys = yp.tile([128, 1024], F32)
                if psum_split:
                    for h in range(2):
                        ph = psy.tile([128, 512], F32)
                        nc.tensor.matmul(ph[:], lhsT2,
                                         t1[:, h * 512:(h + 1) * 512],
                                         start=True, stop=True)
                        nc.vector.transpose(ys[:, h * 512:(h + 1) * 512], ph[:])
                else:
                    py = psy.tile([128, 1024], F32)
                    for h in range(2):
                        nc.tensor.matmul(
                            py[:, h * 512:(h + 1) * 512],
                            lhsT2,
                            t1[:, h * 512:(h + 1) * 512],
                            start=True, stop=True,
                        )
                    if psum_transpose:
                        nc.vector.transpose(ys[:], py[:])
                    else:
                        yc = cpp.tile([128, 1024], F32)
                        nc.scalar.copy(yc[:], py[:])
                        nc.vector.transpose(ys[:], yc[:])

                out_dma(yv[s], as3d(ys[:]))

    nc.compile()
    return nc


def _get_nc(rep=1, use_f32r=False, psum_transpose=False, mode="full"):
    key = (rep, use_f32r, psum_transpose, mode)
    if key not in _NC_CACHE:
        _NC_CACHE[key] = _build_nc(rep=rep, use_f32r=use_f32r,
                                   psum_transpose=psum_transpose, mode=mode)
    return _NC_CACHE[key]


def run_sharded(x, dct_basis, rep=1, use_f32r=False, psum_transpose=False,
                mode="rampopt"):
    """Shard batch over 8 cores, run the Bass kernel SPMD, gather output."""
    from concourse import bass_utils

    x = np.ascontiguousarray(np.asarray(x), dtype=np.float32)
    dct_basis = np.asarray(dct_basis, dtype=np.float32)
    assert x.shape == (B, C, H, W), x.shape

    bt = np.ascontiguousarray(
        np.kron(np.eye(16, dtype=np.float32), dct_basis).T.astype(np.float32)
    )
    bpc = B // N_CORES
    in_maps = [
        {
            "x": x[c * bpc:(c + 1) * bpc].reshape(ROWS_PER_CORE, 512),
            "bt": bt,
        }
        for c in range(N_CORES)
    ]
    if use_f32r:
        for m in in_maps:
            m["btr"] = bt
    if mode in ("fused", "fd"):
        import ml_dtypes
        bth = bt.astype(ml_dtypes.bfloat16)
        btl = (bt - bth.astype(np.float32)).astype(ml_dtypes.bfloat16)
        for m in in_maps:
            m["bth"] = bth
            m["btl"] = btl
    nc = _get_nc(rep=rep, use_f32r=use_f32r, psum_transpose=psum_transpose,
                 mode=mode)
    res = bass_utils.run_bass_kernel_spmd(nc, in_maps, list(range(N_CORES)))
    out = np.concatenate(
        [res.results[c]["y"].reshape(bpc, C, H, W) for c in range(N_CORES)], axis=0
    )
    return out


BEST_MODE = "ff2r"


def kernel(x, dct_basis):
    return run_sharded(x, dct_basis, rep=1, use_f32r=False, mode=BEST_MODE)



# revision 32
# speedup vs baseline: 1.1764x; 1.1764x over previous
"""Trainium2 Bass kernel for batched 8x8-block 2D DCT.

Input  x: (32, 3, 512, 512) f32, dct_basis: (8, 8) f32.
Output y: (32, 3, 512, 512) f32 with each 8x8 block B replaced by D @ B @ D^T.

Sharding: data-parallel over the batch dim - 32 batches -> 8 NeuronCores x 4.
Each core runs an identical (SPMD) Bass program over its (4,3,512,512) slice,
viewed as a [6144, 512] row-major matrix = 24 supertiles of [128, 1024]
(256 image rows x 512 cols; partition p = row within a 128-row band, free
dim = (band t in {0,1}, col w)).

Shipped dataflow (mode "ff2gr" - fused-transpose, all fp32):
  per [128,128] chunk X_c, both DCT passes run on the Tensor engine with the
  DATA as the stationary operand and Bblk^T = kron(I_16, D)^T as the moving
  operand:  PE(out, lhsT=X_c, rhs=Bblk^T) = X_c^T @ Bblk^T = (Bblk X_c)^T.
  Each pass applies the 8-point DCT along the contracted (partition) axis
  AND transposes the chunk, so two passes return to the original
  orientation - no DVE stream transposes and no transpose DMAs at all.
  PSUM->SBUF copies are balanced across the Scalar and Vector engines.
  Input DMAs ride the SP HWDGE ring (latency-sensitive, fast ring);
  output DMAs ride the SWDGE (gpsimd) ring - they are latency-tolerant
  (only buffer recycling waits on them), and this leaves the Scalar engine
  free for copies with no FIFO coupling to the output stream. Data DMAs
  are 1 MiB
  ([128, 2048] supertiles, 4 image-row bands each); a mini-tile ladder at
  the kernel ends shortens pipeline fill/drain. The problem is HBM-bound:
  per core 12 MiB in + 12 MiB out; measured pure-DMA floor ~75-82 us/rep,
  this kernel ~88-91 us steady-state (same-round floor +7-10 us).

Other modes (v2p*, rampopt, dma*, ...) are kept as probe/baseline variants;
see _build_nc.
"""

import sys

for _p in ("/opt/trn_rl_repo",):
    if _p not in sys.path:
        sys.path.insert(0, _p)

from contextlib import ExitStack

import numpy as np

N_CORES = 8
B, C, H, W = 32, 3, 512, 512
ROWS_PER_CORE = (B // N_CORES) * C * H  # 6144
N_SUPER = ROWS_PER_CORE // 256  # 24

_NC_CACHE = {}


def _build_nc(rep=1, use_f32r=False, psum_transpose=False, mode="full"):
    import concourse.bacc as bacc
    import concourse.tile as tile
    import concourse.mybir as mybir

    F32 = mybir.dt.float32
    F32R = mybir.dt.float32r

    FIN = F32R if use_f32r else F32

    nc = bacc.Bacc(
        "TRN2",
        target_bir_lowering=False,
        debug=False,
        enable_asserts=False,
    )
    x_ap = nc.dram_tensor("x", [ROWS_PER_CORE, 512], FIN, kind="ExternalInput").ap()
    bt_ap = nc.dram_tensor("bt", [128, 128], F32, kind="ExternalInput").ap()
    btr_ap = (nc.dram_tensor("btr", [128, 128], F32R, kind="ExternalInput").ap()
              if use_f32r else None)
    BF16 = mybir.dt.bfloat16
    if mode in ("fused", "fd"):
        bth_ap = nc.dram_tensor("bth", [128, 128], BF16, kind="ExternalInput").ap()
        btl_ap = nc.dram_tensor("btl", [128, 128], BF16, kind="ExternalInput").ap()
    y_ap = nc.dram_tensor("y", [ROWS_PER_CORE, 512], F32, kind="ExternalOutput").ap()

    with tile.TileContext(nc) as tc, ExitStack() as ctx:
        xv = x_ap.rearrange("(n t p) w -> n p t w", t=2, p=128)
        yv = y_ap.rearrange("(n t p) w -> n p t w", t=2, p=128)

        def as3d(sb_ap):
            return sb_ap.rearrange("p (t w) -> p t w", t=2)

        const = ctx.enter_context(tc.tile_pool(name="const", bufs=1))
        bt = const.tile([128, 128], F32)
        # constants ride the idle SWDGE ring so the SP HWDGE ring starts on
        # the first data tile immediately
        nc.gpsimd.dma_start(bt[:], bt_ap)
        if use_f32r:
            btr = const.tile([128, 128], F32R)
            nc.gpsimd.dma_start(btr[:], btr_ap)

        nb = 4 if mode in ("tuned", "rampopt", "swin",
                           "v2", "v2p", "v2g", "v3", "v3g", "fd") else 3
        if mode in ("ff", "ffr", "ffx", "ffa"):
            nb = 6
        if mode == "ffb":
            nb = 10
        if mode in ("ff2",):
            nb = 4
        if mode in ("ff2b", "ff2r", "ff2g", "ff2gr", "ff3"):
            nb = 6
        if mode == "ff2gb":
            nb = 8
        if mode in ("dma", "dma2", "dma4", "dmadual", "dmadual0"):
            nb = 6
        if mode == "dmaph":
            nb = N_SUPER
        if mode in ("v2b", "v3b", "v2pb", "v2pg", "v2ps"):
            nb = 6
        if mode == "v2pc":
            nb = 8
        in_dma = (nc.gpsimd.dma_start if mode in ("swin", "v2g", "v3g", "v2pg")
                  else nc.scalar.dma_start if mode in ("v3", "v3b")
                  else nc.sync.dma_start)
        out_dma = (nc.scalar.dma_start
                   if mode in ("tuned", "v2", "v2b", "v2p", "v2pb", "v2pc",
                               "v2pg", "v2ps", "fd", "ff", "ffr", "ffx",
                               "ffb", "ff2", "ff2b", "ff2r", "ffa")
                   else nc.sync.dma_start)
        if mode in ("v2p", "v2pb", "v2pc", "v2pg", "v2ps"):
            psum_transpose = True
        psum_split = mode == "v2ps"
        xp = ctx.enter_context(tc.tile_pool(name="xp", bufs=nb))
        tp = ctx.enter_context(tc.tile_pool(name="tp", bufs=nb))
        yp = ctx.enter_context(tc.tile_pool(name="yp", bufs=nb))
        psb = 4 if mode in ("fused", "fd") else 2
        pst = ctx.enter_context(tc.tile_pool(name="pst", bufs=psb, space="PSUM"))
        psy = ctx.enter_context(tc.tile_pool(name="psy", bufs=psb, space="PSUM"))
        cpp = ctx.enter_context(tc.tile_pool(name="cpp", bufs=nb))

        lhsT1 = btr[:] if use_f32r else bt[:]
        lhsT2 = bt[:]

        if mode in ("fused", "fd"):
            # Fused-transpose dataflow: data chunks are the STATIONARY
            # operand (fp32, full precision); the moving operand is the
            # basis split hi/lo into bf16 (1 cyc/row) and accumulated in
            # PSUM: out = X_c^T @ (Bth + Btl). Two such matmul pairs per
            # chunk implement both DCT passes with the transposes absorbed
            # by lhsT.T semantics. No DVE stream transposes needed.
            bth = const.tile([128, 128], BF16)
            nc.gpsimd.dma_start(bth[:], bth_ap)
            btl = const.tile([128, 128], BF16)
            nc.gpsimd.dma_start(btl[:], btl_ap)
            for _ in range(rep):
                for s in range(N_SUPER):
                    xs = xp.tile([128, 1024], F32)
                    in_dma(as3d(xs[:]), xv[s])
                    t1 = tp.tile([128, 1024], F32)
                    for b in range(2):
                        pt = pst.tile([128, 512], F32)
                        for q in range(4):
                            c = b * 4 + q
                            for rhs_t, st in ((bth, True), (btl, False)):
                                nc.tensor.matmul(
                                    pt[:, q * 128:(q + 1) * 128],
                                    xs[:, c * 128:(c + 1) * 128],
                                    rhs_t[:],
                                    start=st, stop=not st,
                                    skip_group_check=True,
                                )
                        nc.scalar.copy(t1[:, b * 512:(b + 1) * 512], pt[:])
                    ys = yp.tile([128, 1024], F32)
                    for b in range(2):
                        py = psy.tile([128, 512], F32)
                        for q in range(4):
                            c = b * 4 + q
                            for rhs_t, st in ((bth, True), (btl, False)):
                                nc.tensor.matmul(
                                    py[:, q * 128:(q + 1) * 128],
                                    t1[:, c * 128:(c + 1) * 128],
                                    rhs_t[:],
                                    start=st, stop=not st,
                                    skip_group_check=True,
                                )
                        if b == 0:
                            nc.scalar.copy(ys[:, :512], py[:])
                        else:
                            nc.vector.tensor_copy(ys[:, 512:], py[:])
                    out_dma(yv[s], as3d(ys[:]))
            rep = 0  # skip main loop below

        if mode in ("ff", "ffr", "ffx", "ffb", "ffa"):
            # Fused-transpose dataflow, all-fp32: data chunks [128,128] are
            # the STATIONARY operand, the basis bt (kron(I,D)^T) the MOVING
            # operand for BOTH passes. out = X_c^T @ bt transposes each
            # chunk while applying the DCT along the contracted axis, so the
            # two passes come back to the original orientation with no DVE
            # stream transposes. PSUM->SBUF copies balanced ACT/DVE.
            # ffr: + mini-tile ladder at the kernel ends (shorter fill/drain)
            # ffx: ffr + in/out rings alternate per supertile
            def ff_full(s, ind, outd):
                xs = xp.tile([128, 1024], F32)
                ind(as3d(xs[:]), xv[s])
                t1 = tp.tile([128, 1024], F32)
                for b in range(2):
                    pt = pst.tile([128, 512], F32)
                    for q in range(4):
                        c = b * 4 + q
                        nc.tensor.matmul(
                            pt[:, q * 128:(q + 1) * 128],
                            xs[:, c * 128:(c + 1) * 128],
                            bt[:], start=True, stop=True,
                        )
                    sl = t1[:, b * 512:(b + 1) * 512]
                    if mode == "ffa":
                        # all t1 copies on DVE; ACT reserved for ys + out-DMA
                        nc.vector.tensor_copy(sl, pt[:])
                    elif b == 0:
                        nc.scalar.copy(sl, pt[:])
                    else:
                        nc.vector.tensor_copy(sl, pt[:])
                ys = yp.tile([128, 1024], F32)
                for b in range(2):
                    py = psy.tile([128, 512], F32)
                    for q in range(4):
                        c = b * 4 + q
                        nc.tensor.matmul(
                            py[:, q * 128:(q + 1) * 128],
                            t1[:, c * 128:(c + 1) * 128],
                            bt[:], start=True, stop=True,
                        )
                    sl = ys[:, b * 512:(b + 1) * 512]
                    if mode == "ffa":
                        # ys copies on ACT so out-DMA's wait is queue-local
                        nc.scalar.copy(sl, py[:])
                    elif b == 0:
                        nc.scalar.copy(sl, py[:])
                    else:
                        nc.vector.tensor_copy(sl, py[:])
                outd(yv[s], as3d(ys[:]))

            def ff_mini(s, t, w0, w1, ind, outd):
                wd = w1 - w0
                xs = xp.tile([128, wd], F32)
                ind(xs[:], xv[s][:, t, w0:w1])
                t1 = tp.tile([128, wd], F32)
                pt = pst.tile([128, wd], F32)
                for q in range(wd // 128):
                    nc.tensor.matmul(
                        pt[:, q * 128:(q + 1) * 128],
                        xs[:, q * 128:(q + 1) * 128],
                        bt[:], start=True, stop=True,
                    )
                nc.scalar.copy(t1[:], pt[:])
                ys = yp.tile([128, wd], F32)
                py = psy.tile([128, wd], F32)
                for q in range(wd // 128):
                    nc.tensor.matmul(
                        py[:, q * 128:(q + 1) * 128],
                        t1[:, q * 128:(q + 1) * 128],
                        bt[:], start=True, stop=True,
                    )
                nc.vector.tensor_copy(ys[:], py[:])
                outd(yv[s][:, t, w0:w1], ys[:])

            ff_ladder = mode in ("ffr", "ffx")
            for r in range(rep):
                for s in range(N_SUPER):
                    if mode == "ffx" and s % 2:
                        ind, outd = nc.scalar.dma_start, nc.sync.dma_start
                    else:
                        ind, outd = in_dma, out_dma
                    at_end = ((r == 0 and s <= 1)
                              or (r == rep - 1 and s >= N_SUPER - 2))
                    if ff_ladder and at_end:
                        if s in (0, N_SUPER - 1):
                            for t in range(2):
                                for qq in range(2):
                                    ff_mini(s, t, qq * 256, (qq + 1) * 256,
                                            ind, outd)
                        else:
                            for t in range(2):
                                ff_mini(s, t, 0, 512, ind, outd)
                    else:
                        ff_full(s, ind, outd)
            rep = 0  # skip main loop below

        if mode in ("ff2", "ff2b", "ff2r", "ff2g", "ff2gr",
                    "ff3", "ff2gb"):
            # ff dataflow with 1 MiB transfers: [128, 2048] supertiles
            # ff2g/ff2gr: out-DMAs ride the SWDGE (gpsimd) ring so ACT only
            # does copies - no FIFO coupling between copies and the
            # latency-tolerant output stream; SP carries only loads.
            # (4 bands), per-band fused two-pass chunks, copies ACT/DVE
            # alternating per band. ff2r adds the end ladder.
            xv4 = x_ap.rearrange("(n t p) w -> n p t w", t=4, p=128)
            yv4 = y_ap.rearrange("(n t p) w -> n p t w", t=4, p=128)

            def ff2_full(sp, ind):
                xs = xp.tile([128, 2048], F32)
                ind(xs[:].rearrange("p (t w) -> p t w", t=4), xv4[sp])
                t1 = tp.tile([128, 2048], F32)
                for g in range(4):
                    pt = pst.tile([128, 512], F32)
                    for q in range(4):
                        c = g * 4 + q
                        nc.tensor.matmul(
                            pt[:, q * 128:(q + 1) * 128],
                            xs[:, c * 128:(c + 1) * 128],
                            bt[:], start=True, stop=True,
                        )
                    if g % 2 == 0:
                        nc.scalar.copy(t1[:, g * 512:(g + 1) * 512], pt[:])
                    else:
                        nc.vector.tensor_copy(
                            t1[:, g * 512:(g + 1) * 512], pt[:])
                ys = yp.tile([128, 2048], F32)
                for g in range(4):
                    py = psy.tile([128, 512], F32)
                    for q in range(4):
                        c = g * 4 + q
                        nc.tensor.matmul(
                            py[:, q * 128:(q + 1) * 128],
                            t1[:, c * 128:(c + 1) * 128],
                            bt[:], start=True, stop=True,
                        )
                    if g % 2 == 0:
                        nc.scalar.copy(ys[:, g * 512:(g + 1) * 512], py[:])
                    else:
                        nc.vector.tensor_copy(
                            ys[:, g * 512:(g + 1) * 512], py[:])
                outd2(yv4[sp], ys[:].rearrange("p (t w) -> p t w", t=4))

            def ff2_mini(sp, t, half, ind):
                # one [128, 512] band-half of 2-band supertile sp
                s2, tt = divmod(sp * 4 + t, 2)
                xs = xp.tile([128, 512], F32)
                ind(xs[:], xv[s2][:, tt, :])
                t1 = tp.tile([128, 512], F32)
                pt = pst.tile([128, 512], F32)
                for q in range(4):
                    nc.tensor.matmul(
                        pt[:, q * 128:(q + 1) * 128],
                        xs[:, q * 128:(q + 1) * 128],
                        bt[:], start=True, stop=True,
                    )
                nc.scalar.copy(t1[:], pt[:])
                ys = yp.tile([128, 512], F32)
                py = psy.tile([128, 512], F32)
                for q in range(4):
                    nc.tensor.matmul(
                        py[:, q * 128:(q + 1) * 128],
                        t1[:, q * 128:(q + 1) * 128],
                        bt[:], start=True, stop=True,
                    )
                nc.vector.tensor_copy(ys[:], py[:])
                outd2(yv[s2][:, tt, :], ys[:])

            outd2 = (nc.gpsimd.dma_start
                     if mode in ("ff2g", "ff2gr", "ff3", "ff2gb")
                     else out_dma)
            NSP = N_SUPER // 2  # 12 supertiles of [128, 2048]
            for r in range(rep):
                for sp in range(NSP):
                    ind = (nc.scalar.dma_start
                           if mode == "ff3" and sp % 2 else in_dma)
                    at_end = ((r == 0 and sp == 0)
                              or (r == rep - 1 and sp == NSP - 1))
                    if mode in ("ff2r", "ff2gr", "ff3", "ff2gb") and at_end:
                        for t in range(4):
                            ff2_mini(sp, t, 0, ind)
                    else:
                        ff2_full(sp, ind)
            rep = 0  # skip main loop below

        if mode == "bigload":
            # steady-state probe: 1 MiB input DMAs (two supertiles per load),
            # compute pipeline and 512 KiB output DMAs unchanged
            xv4 = x_ap.rearrange("(n t p) w -> n p t w", t=4, p=128)
            for _ in range(rep):
                for sp in range(N_SUPER // 2):
                    xs2 = xp.tile([128, 2048], F32)
                    nc.sync.dma_start(
                        xs2[:].rearrange("p (t w) -> p t w", t=4), xv4[sp])
                    for g in range(2):
                        s = sp * 2 + g
                        xsv = xs2[:, g * 1024:(g + 1) * 1024]
                        pt = pst.tile([128, 1024], F32)
                        for h in range(2):
                            nc.tensor.matmul(
                                pt[:, h * 512:(h + 1) * 512], lhsT2,
                                xsv[:, h * 512:(h + 1) * 512],
                                start=True, stop=True)
                        tc_ = cpp.tile([128, 1024], F32)
                        nc.scalar.copy(tc_[:], pt[:])
                        t1 = tp.tile([128, 1024], F32)
                        nc.vector.transpose(t1[:], tc_[:])
                        py = psy.tile([128, 1024], F32)
                        for h in range(2):
                            nc.tensor.matmul(
                                py[:, h * 512:(h + 1) * 512], lhsT2,
                                t1[:, h * 512:(h + 1) * 512],
                                start=True, stop=True)
                        yc = cpp.tile([128, 1024], F32)
                        nc.scalar.copy(yc[:], py[:])
                        ys = yp.tile([128, 1024], F32)
                        nc.vector.transpose(ys[:], yc[:])
                        nc.sync.dma_start(yv[s], as3d(ys[:]))
            rep = 0  # skip main loop below

        if mode == "full2":
            # [128, 2048] supertiles: 1 MiB DMA transfers, compute in
            # [128, 1024] halves (PSUM: 2+2 banks x2 pools = 8 banks).
            xv4 = x_ap.rearrange("(n t p) w -> n p t w", t=4, p=128)
            yv4 = y_ap.rearrange("(n t p) w -> n p t w", t=4, p=128)
            for _ in range(rep):
                for s in range(N_SUPER // 2):
                    xs = xp.tile([128, 2048], FIN)
                    nc.sync.dma_start(
                        xs[:].rearrange("p (t w) -> p t w", t=4), xv4[s])
                    ys = yp.tile([128, 2048], F32)
                    for g in range(2):
                        pt = pst.tile([128, 1024], F32)
                        for h in range(2):
                            nc.tensor.matmul(
                                pt[:, h * 512:(h + 1) * 512],
                                lhsT1,
                                xs[:, g * 1024 + h * 512:
                                   g * 1024 + (h + 1) * 512],
                                start=True, stop=True,
                            )
                        tc_ = cpp.tile([128, 1024], F32)
                        nc.scalar.copy(tc_[:], pt[:])
                        t1 = tp.tile([128, 1024], F32)
                        nc.vector.transpose(t1[:], tc_[:])
                        py = psy.tile([128, 1024], F32)
                        for h in range(2):
                            nc.tensor.matmul(
                                py[:, h * 512:(h + 1) * 512],
                                lhsT2,
                                t1[:, h * 512:(h + 1) * 512],
                                start=True, stop=True,
                            )
                        yc = cpp.tile([128, 1024], F32)
                        nc.scalar.copy(yc[:], py[:])
                        nc.vector.transpose(
                            ys[:, g * 1024:(g + 1) * 1024], yc[:])
                    nc.sync.dma_start(
                        yv4[s], ys[:].rearrange("p (t w) -> p t w", t=4))
            rep = 0  # skip main loop below

        if mode == "dma2":
            # 1 MiB transfers: [128, 2048] supertiles (4 bands each)
            xv4 = x_ap.rearrange("(n t p) w -> n p t w", t=4, p=128)
            yv4 = y_ap.rearrange("(n t p) w -> n p t w", t=4, p=128)
            for _ in range(rep):
                for s in range(N_SUPER // 2):
                    xs = xp.tile([128, 2048], FIN)
                    nc.sync.dma_start(
                        xs[:].rearrange("p (t w) -> p t w", t=4), xv4[s])
                    nc.sync.dma_start(
                        yv4[s], xs[:].rearrange("p (t w) -> p t w", t=4))
            rep = 0  # skip main loop below

        if mode == "dma4":
            # 2 MiB transfers: [128, 4096] supertiles (8 bands each)
            xv8 = x_ap.rearrange("(n t p) w -> n p t w", t=8, p=128)
            yv8 = y_ap.rearrange("(n t p) w -> n p t w", t=8, p=128)
            for _ in range(rep):
                for s in range(N_SUPER // 4):
                    xs = xp.tile([128, 4096], FIN)
                    nc.sync.dma_start(
                        xs[:].rearrange("p (t w) -> p t w", t=8), xv8[s])
                    nc.sync.dma_start(
                        yv8[s], xs[:].rearrange("p (t w) -> p t w", t=8))
            rep = 0  # skip main loop below

        if mode == "dmadual":
            # 1 MiB transfers, in on SP HWDGE ring, out on ACT HWDGE ring
            xv4 = x_ap.rearrange("(n t p) w -> n p t w", t=4, p=128)
            yv4 = y_ap.rearrange("(n t p) w -> n p t w", t=4, p=128)
            for _ in range(rep):
                for s in range(N_SUPER // 2):
                    xs = xp.tile([128, 2048], FIN)
                    nc.sync.dma_start(
                        xs[:].rearrange("p (t w) -> p t w", t=4), xv4[s])
                    nc.scalar.dma_start(
                        yv4[s], xs[:].rearrange("p (t w) -> p t w", t=4))
            rep = 0  # skip main loop below

        if mode == "dmadual0":
            # 512 KiB transfers, in on SP ring, out on ACT ring
            for _ in range(rep):
                for s in range(N_SUPER):
                    xs = xp.tile([128, 1024], FIN)
                    nc.sync.dma_start(as3d(xs[:]), xv[s])
                    nc.scalar.dma_start(yv[s], as3d(xs[:]))
            rep = 0  # skip main loop below

        if mode == "dmaph":
            # phase-separated: read all 24 supertiles into SBUF, then write
            # all 24 back — measures HBM R/W turnaround cost vs interleaved
            for _ in range(rep):
                tiles = []
                for s in range(N_SUPER):
                    xs = xp.tile([128, 1024], FIN)
                    nc.sync.dma_start(as3d(xs[:]), xv[s])
                    tiles.append(xs)
                for s in range(N_SUPER):
                    nc.scalar.dma_start(yv[s], as3d(tiles[s][:]))
            rep = 0  # skip main loop below

        def mini_super(s, t, w0, w1):
            # [128, w1-w0] slice of band t as its own mini-pipeline; used at
            # the kernel ends to shorten pipeline fill and drain
            wd = w1 - w0
            xs = xp.tile([128, wd], FIN)
            in_dma(xs[:], xv[s][:, t, w0:w1])
            pt = pst.tile([128, wd], F32)
            nc.tensor.matmul(pt[:], lhsT1, xs[:], start=True, stop=True)
            tc_ = cpp.tile([128, wd], F32)
            nc.scalar.copy(tc_[:], pt[:])
            t1 = tp.tile([128, wd], F32)
            nc.vector.transpose(t1[:], tc_[:])
            py = psy.tile([128, wd], F32)
            nc.tensor.matmul(py[:], lhsT2, t1[:], start=True, stop=True)
            yc = cpp.tile([128, wd], F32)
            nc.scalar.copy(yc[:], py[:])
            ys = yp.tile([128, wd], F32)
            nc.vector.transpose(ys[:], yc[:])
            out_dma(yv[s][:, t, w0:w1], ys[:])

        # granularity ladder per supertile index: list of (t, w0, w1) items,
        # or None for the standard full-width path
        def ladder(s):
            if s in (0, N_SUPER - 1):
                items = [(t, q * 256, (q + 1) * 256)
                         for t in range(2) for q in range(2)]
                return items
            if s in (1, N_SUPER - 2):
                return [(0, 0, 512), (1, 0, 512)]
            return None

        split_ends = mode in ("rampopt", "swin", "v2", "v2b", "v2p", "v2g",
                              "v3", "v3b", "v3g", "v2pb", "v2pc", "v2pg",
                              "v2ps")
        for r in range(rep):
            for s in range(N_SUPER):
                # ladder only at the true kernel ends (first/last rep), so
                # rep>1 timing builds measure pure steady-state in between;
                # for rep=1 this is the same program as before
                at_end = (r == 0 and s <= 1) or (r == rep - 1 and s >= N_SUPER - 2)
                items = ladder(s) if (split_ends and at_end) else None
                if items is not None:
                    for (t, w0, w1) in items:
                        mini_super(s, t, w0, w1)
                    continue
                xs = xp.tile([128, 1024], FIN)
                in_dma(as3d(xs[:]), xv[s])

                if mode == "dma":
                    nc.sync.dma_start(yv[s], as3d(xs[:]))
                    continue

                t1 = tp.tile([128, 1024], F32)
                if psum_split:
                    for h in range(2):
                        ph = pst.tile([128, 512], F32)
                        nc.tensor.matmul(ph[:], lhsT1,
                                         xs[:, h * 512:(h + 1) * 512],
                                         start=True, stop=True)
                        nc.vector.transpose(t1[:, h * 512:(h + 1) * 512], ph[:])
                else:
                    pt = pst.tile([128, 1024], F32)
                    for h in range(2):
                        nc.tensor.matmul(
                            pt[:, h * 512:(h + 1) * 512],
                            lhsT1,
                            xs[:, h * 512:(h + 1) * 512],
                            start=True, stop=True,
                        )
                    if psum_transpose:
                        nc.vector.transpose(t1[:], pt[:])
                    else:
                        tc_ = cpp.tile([128, 1024], F32)
                        nc.scalar.copy(tc_[:], pt[:])
                        nc.vector.transpose(t1[:], tc_[:])

                ys = yp.tile([128, 1024], F32)
                if psum_split:
                    for h in range(2):
                        ph = psy.tile([128, 512], F32)
                        nc.tensor.matmul(ph[:], lhsT2,
                                         t1[:, h * 512:(h + 1) * 512],
                                         start=True, stop=True)
                        nc.vector.transpose(ys[:, h * 512:(h + 1) * 512], ph[:])
                else:
                    py = psy.tile([128, 1024], F32)
                    for h in range(2):
                        nc.tensor.matmul(
                            py[:, h * 512:(h + 1) * 512],
                            lhsT2,
                            t1[:, h * 512:(h + 1) * 512],
                            start=True, stop=True,
                        )
                    if psum_transpose:
                        nc.vector.transpose(ys[:], py[:])
                    else:
                        yc = cpp.tile([128, 1024], F32)
                        nc.scalar.copy(yc[:], py[:])
                        nc.vector.transpose(ys[:], yc[:])

                out_dma(yv[s], as3d(ys[:]))

    nc.compile()
    return nc


def _get_nc(rep=1, use_f32r=False, psum_transpose=False, mode="full"):
    key = (rep, use_f32r, psum_transpose, mode)
    if key not in _NC_CACHE:
        _NC_CACHE[key] = _build_nc(rep=rep, use_f32r=use_f32r,
                                   psum_transpose=psum_transpose, mode=mode)
    return _NC_CACHE[key]


def run_sharded(x, dct_basis, rep=1, use_f32r=False, psum_transpose=False,
                mode="rampopt"):
    """Shard batch over 8 cores, run the Bass kernel SPMD, gather output."""
    from concourse import bass_utils

    x = np.ascontiguousarray(np.asarray(x), dtype=np.float32)
    dct_basis = np.asarray(dct_basis, dtype=np.float32)
    assert x.shape == (B, C, H, W), x.shape

    bt = np.ascontiguousarray(
        np.kron(np.eye(16, dtype=np.float32), dct_basis).T.astype(np.float32)
    )
    bpc = B // N_CORES
    in_maps = [
        {
            "x": x[c * bpc:(c + 1) * bpc].reshape(ROWS_PER_CORE, 512),
            "bt": bt,
        }
        for c in range(N_CORES)
    ]
    if use_f32r:
        for m in in_maps:
            m["btr"] = bt
    if mode in ("fused", "fd"):
        import ml_dtypes
        bth = bt.astype(ml_dtypes.bfloat16)
        btl = (bt - bth.astype(np.float32)).astype(ml_dtypes.bfloat16)
        for m in in_maps:
            m["bth"] = bth
            m["btl"] = btl
    nc = _get_nc(rep=rep, use_f32r=use_f32r, psum_transpose=psum_transpose,
                 mode=mode)
    res = bass_utils.run_bass_kernel_spmd(nc, in_maps, list(range(N_CORES)))
    out = np.concatenate(
        [res.results[c]["y"].reshape(bpc, C, H, W) for c in range(N_CORES)], axis=0
    )
    return out


BEST_MODE = "ff2gr"


def kernel(x, dct_basis):
    return run_sharded(x, dct_basis, rep=1, use_f32r=False, mode=BEST_MODE)

